# revision 17
# baseline (speedup 1.0000x reference)
"""CompressedLinear (int8 weight, per-row scale) on 8 Trainium2 NeuronCores.

Math: y[b,s,o] = sum_i x[b,s,i] * (w_int8[o,i] * scale[o]) + bias[o]

Strategy (tensor-parallel over out_features, per sharding hint):
  - Shard W/scale/bias rows across 8 cores (1376 rows each); x replicated.
  - Scale is applied to the matmul OUTPUT (algebraically identical), so the
    device matmuls run on the raw int8 weights cast to bf16 (int8 is exact
    in bf16).
  - x (f32) is split on-device into bf16 hi + lo parts (x = hi + lo with
    |x - hi - lo| <= 2^-16|x|), giving ~f32 accuracy from two bf16 matmul
    passes that accumulate into the same f32 PSUM tile - ~2x faster than
    fp32 matmuls on the PE.
  - Each core computes yT[o_shard, s] = W_shard @ x^T; both operands need
    the contraction dim on SBUF partitions, so the host hands each core
    pre-transposed views (pure layout permutation done while sharding):
    xt = x^T [4096, 2048] f32 and wt = W_shard^T [4096, 1376] int8.
  - Per-partition affine (scale, bias) is fused into the PSUM eviction.
"""

import os
import numpy as np

import concourse.bass as bass
import concourse.tile as tile
from concourse import bacc, mybir
from concourse.bass_utils import run_bass_kernel_spmd

B = 1
S = 2048
I = 4096
O = 11008
N_CORES = 8
O_SHARD = O // N_CORES  # 1376
S_CHUNK = 512
P = 128


def build_bass(I_=I, O_SHARD_=O_SHARD, S_=S, S_CHUNK_=S_CHUNK):
    KT = I_ // P
    N_CHUNKS = S_ // S_CHUNK_
    OT = (O_SHARD_ + P - 1) // P
    full_t = O_SHARD_ // P
    rem = O_SHARD_ - full_t * P

    nc = bacc.Bacc("TRN2", target_bir_lowering=False, debug=False)

    xt = nc.dram_tensor("xt", [I_, S_], mybir.dt.float32, kind="ExternalInput").ap()
    wt = nc.dram_tensor("wt", [I_, O_SHARD_], mybir.dt.int8, kind="ExternalInput").ap()
    scale = nc.dram_tensor("scale", [O_SHARD_], mybir.dt.float32, kind="ExternalInput").ap()
    bias = nc.dram_tensor("bias", [O_SHARD_], mybir.dt.float32, kind="ExternalInput").ap()
    yt = nc.dram_tensor("yt", [O_SHARD_, S_], mybir.dt.float32, kind="ExternalOutput").ap()

    with tile.TileContext(nc) as tc:
        with (
            tc.tile_pool(name="wres", bufs=1) as wres_pool,
            tc.tile_pool(name="consts", bufs=1) as const_pool,
            tc.tile_pool(name="xstage", bufs=4) as xstage_pool,
            tc.tile_pool(name="xhilo", bufs=min(KT + 8, KT * N_CHUNKS)) as xhilo_pool,
            tc.tile_pool(name="outp", bufs=4) as out_pool,
            tc.tile_pool(name="psum", bufs=8, space="PSUM") as psum_pool,
        ):
            # Weight shard int8 -> bf16, kept resident in SBUF. One tile per
            # k-slice so matmuls only depend on their own slice. The
            # int8->bf16 cast happens inside the DMA (SWDGE path), so no
            # compute engine spends time on it. Tiles are emitted interleaved
            # with the first chunk's x loads (see below) so kt=0 completes
            # first and matmuls start as early as possible.
            w_res = [None] * KT

            def emit_w(kt):
                w_kt = wres_pool.tile([P, O_SHARD_], mybir.dt.bfloat16, tag=f"w{kt}")
                nc.gpsimd.dma_start(w_kt[:], wt[kt * P:(kt + 1) * P, :])
                w_res[kt] = w_kt

            # per-partition scale/bias columns: [p, t] = value for o = t*128 + p
            scale_t = const_pool.tile([P, OT], mybir.dt.float32)
            bias_t = const_pool.tile([P, OT], mybir.dt.float32)
            if full_t:
                nc.sync.dma_start(
                    scale_t[:, :full_t], scale[: full_t * P].rearrange("(t p) -> p t", p=P)
                )
                nc.sync.dma_start(
                    bias_t[:, :full_t], bias[: full_t * P].rearrange("(t p) -> p t", p=P)
                )
            if rem:
                nc.sync.dma_start(
                    scale_t[:rem, full_t:], scale[full_t * P:].rearrange("(t p) -> p t", p=rem)
                )
                nc.sync.dma_start(
                    bias_t[:rem, full_t:], bias[full_t * P:].rearrange("(t p) -> p t", p=rem)
                )

            # PSUM bank groups: 4+4+3 o-tiles so two adjacent groups fit in
            # the 8 banks and group transitions never wait on drains.
            groups = []
            g0 = 0
            for gsz in (4, 4, 3):
                if g0 < OT:
                    groups.append((g0, min(g0 + gsz, OT)))
                    g0 += gsz

            def emit_conversions(sc):
                s0 = sc * S_CHUNK_
                his, los = [], []
                for kt in range(KT):
                    if sc == 0:
                        emit_w(kt)
                    xstage = xstage_pool.tile([P, S_CHUNK_], mybir.dt.float32)
                    nc.sync.dma_start(xstage[:], xt[kt * P:(kt + 1) * P, s0:s0 + S_CHUNK_])
                    xhi = xhilo_pool.tile([P, S_CHUNK_], mybir.dt.bfloat16, tag="xhi")
                    nc.vector.tensor_copy(xhi[:], xstage[:])
                    xlo = xhilo_pool.tile([P, S_CHUNK_], mybir.dt.bfloat16, tag="xlo")
                    nc.vector.tensor_sub(xlo[:], xstage[:], xhi[:])
                    his.append(xhi)
                    los.append(xlo)
                return his, los

            def emit_groups(sc, his, los):
                # kt outer / o-tile inner: each x tile's last reader comes
                # early in the group sweep, so next-chunk conversions spread
                # over the whole chunk instead of bunching at its tail.
                s0 = sc * S_CHUNK_
                for g_start, g_end in groups:
                    psums = {}
                    for ot in range(g_start, g_end):
                        psums[ot] = psum_pool.tile(
                            [P, S_CHUNK_], mybir.dt.float32,
                            name=f"psum_{sc}_{ot}", tag="psum",
                        )
                    for kt in range(KT):
                        for ot in range(g_start, g_end):
                            orows = min(P, O_SHARD_ - ot * P)
                            w_slice = w_res[kt][:, ot * P:ot * P + orows]
                            nc.tensor.matmul(
                                psums[ot][:orows, :], w_slice, his[kt][:],
                                start=(kt == 0), stop=False,
                            )
                            nc.tensor.matmul(
                                psums[ot][:orows, :], w_slice, los[kt][:],
                                start=False, stop=(kt == KT - 1),
                            )
                    for ot in range(g_start, g_end):
                        orows = min(P, O_SHARD_ - ot * P)
                        out_t = out_pool.tile([P, S_CHUNK_], mybir.dt.float32)
                        nc.vector.tensor_scalar(
                            out=out_t[:orows, :],
                            in0=psums[ot][:orows, :],
                            scalar1=scale_t[:orows, ot:ot + 1],
                            scalar2=bias_t[:orows, ot:ot + 1],
                            op0=mybir.AluOpType.mult,
                            op1=mybir.AluOpType.add,
                        )
                        nc.sync.dma_start(
                            yt[ot * P:ot * P + orows, s0:s0 + S_CHUNK_],
                            out_t[:orows, :],
                        )

            # Software-pipelined emission: conversions for chunk sc+1 are
            # emitted before chunk sc's matmul groups, so in the per-engine
            # FIFO streams next-chunk subs/casts sit ahead of this chunk's
            # PSUM drains.
            prev = emit_conversions(0)
            for sc in range(N_CHUNKS):
                if sc + 1 < N_CHUNKS:
                    nxt = emit_conversions(sc + 1)
                else:
                    nxt = None
                emit_groups(sc, *prev)
                prev = nxt

    nc.compile()
    return nc


_NC_CACHE = None


def _get_nc():
    global _NC_CACHE
    if _NC_CACHE is None:
        _NC_CACHE = build_bass()
    return _NC_CACHE


def run(inputs, trace=False, trace_cores=None, tmpdir=None):
    x = np.asarray(inputs["x"])
    w = np.asarray(inputs["weight_int8"])
    scale = np.asarray(inputs["scale"], dtype=np.float32)
    bias = np.asarray(inputs["bias"], dtype=np.float32)

    if w.dtype != np.int8:
        w = w.astype(np.int8)
    x2d = np.ascontiguousarray(x.reshape(S, I).astype(np.float32, copy=False))
    xtr = np.ascontiguousarray(x2d.T)  # [I, S]

    in_maps = []
    for c in range(N_CORES):
        sl = slice(c * O_SHARD, (c + 1) * O_SHARD)
        in_maps.append({
            "xt": xtr,
            "wt": np.ascontiguousarray(w[sl, :].T),  # [I, O_SHARD]
            "scale": np.ascontiguousarray(scale[sl]),
            "bias": np.ascontiguousarray(bias[sl]),
        })

    nc = _get_nc()
    kwargs = {}
    if trace:
        kwargs["trace"] = True
        if trace_cores is not None:
            kwargs["trace_cores"] = trace_cores
        if tmpdir is not None:
            kwargs["tmpdir"] = tmpdir
    res = run_bass_kernel_spmd(nc, in_maps, core_ids=list(range(N_CORES)), **kwargs)

    yt_full = np.concatenate([res.results[c]["yt"] for c in range(N_CORES)], axis=0)
    out = np.ascontiguousarray(yt_full.T).reshape(B, S, O).astype(np.float32, copy=False)
    if trace:
        return out, res
    return out


def kernel(**inputs) -> np.ndarray:
    return run(inputs, trace=False)


# revision 19
# speedup vs baseline: 1.0051x; 1.0051x over previous
"""CompressedLinear (int8 weight, per-row scale) on 8 Trainium2 NeuronCores.

Math: y[b,s,o] = sum_i x[b,s,i] * (w_int8[o,i] * scale[o]) + bias[o]

Strategy (tensor-parallel over out_features, per sharding hint):
  - Shard W/scale/bias rows across 8 cores (1376 rows each); x replicated.
  - Scale is applied to the matmul OUTPUT (algebraically identical), so the
    device matmuls run on the raw int8 weights cast to bf16 (int8 is exact
    in bf16).
  - x (f32) is split on-device into bf16 hi + lo parts (x = hi + lo with
    |x - hi - lo| <= 2^-16|x|), giving ~f32 accuracy from two bf16 matmul
    passes that accumulate into the same f32 PSUM tile - ~2x faster than
    fp32 matmuls on the PE.
  - Each core computes yT[o_shard, s] = W_shard @ x^T; both operands need
    the contraction dim on SBUF partitions, so the host hands each core
    pre-transposed views (pure layout permutation done while sharding):
    xt = x^T [4096, 2048] f32 and wt = W_shard^T [4096, 1376] int8.
  - Per-partition affine (scale, bias) is fused into the PSUM eviction.
"""

import os
import numpy as np

import concourse.bass as bass
import concourse.tile as tile
from concourse import bacc, mybir
from concourse.bass_utils import run_bass_kernel_spmd

B = 1
S = 2048
I = 4096
O = 11008
N_CORES = 8
O_SHARD = O // N_CORES  # 1376
S_CHUNK = 512
P = 128


def build_bass(I_=I, O_SHARD_=O_SHARD, S_=S, S_CHUNK_=S_CHUNK):
    KT = I_ // P
    N_CHUNKS = S_ // S_CHUNK_
    OT = (O_SHARD_ + P - 1) // P
    full_t = O_SHARD_ // P
    rem = O_SHARD_ - full_t * P

    nc = bacc.Bacc("TRN2", target_bir_lowering=False, debug=False)

    xt = nc.dram_tensor("xt", [I_, S_], mybir.dt.float32, kind="ExternalInput").ap()
    wt = nc.dram_tensor("wt", [I_, O_SHARD_], mybir.dt.int8, kind="ExternalInput").ap()
    scale = nc.dram_tensor("scale", [O_SHARD_], mybir.dt.float32, kind="ExternalInput").ap()
    bias = nc.dram_tensor("bias", [O_SHARD_], mybir.dt.float32, kind="ExternalInput").ap()
    yt = nc.dram_tensor("yt", [O_SHARD_, S_], mybir.dt.float32, kind="ExternalOutput").ap()

    with tile.TileContext(nc) as tc:
        with (
            tc.tile_pool(name="wres", bufs=1) as wres_pool,
            tc.tile_pool(name="consts", bufs=1) as const_pool,
            tc.tile_pool(name="xstage", bufs=4) as xstage_pool,
            tc.tile_pool(name="xhilo", bufs=min(KT + 8, KT * N_CHUNKS)) as xhilo_pool,
            tc.tile_pool(name="outp", bufs=4) as out_pool,
            tc.tile_pool(name="psum", bufs=8, space="PSUM") as psum_pool,
        ):
            # Weight shard int8 -> bf16, kept resident in SBUF. One tile per
            # k-slice so matmuls only depend on their own slice. The
            # int8->bf16 cast happens inside the DMA (SWDGE path), so no
            # compute engine spends time on it. Tiles are emitted interleaved
            # with the first chunk's x loads (see below) so kt=0 completes
            # first and matmuls start as early as possible.
            w_res = [None] * KT

            def emit_w(kt):
                w_kt = wres_pool.tile([P, O_SHARD_], mybir.dt.bfloat16, tag=f"w{kt}")
                nc.gpsimd.dma_start(w_kt[:], wt[kt * P:(kt + 1) * P, :])
                w_res[kt] = w_kt

            # PE warm-up: ~36 dependency-free matmuls on a zeroed tile keep
            # the PE busy during the initial DMA window, so the HAM clock
            # gate opens (K=8/8) before the first real matmul issues.
            warm_sb = const_pool.tile([P, P], mybir.dt.bfloat16)
            nc.any.memset(warm_sb[:], 0.0)
            warm_ps = psum_pool.tile([P, P], mybir.dt.float32, name="warm_ps", tag="psum")
            N_WARM = 36
            for i in range(N_WARM):
                nc.tensor.matmul(
                    warm_ps[:], warm_sb[:], warm_sb[:],
                    start=(i == 0), stop=(i == N_WARM - 1),
                )

            # per-partition scale/bias columns: [p, t] = value for o = t*128 + p
            scale_t = const_pool.tile([P, OT], mybir.dt.float32)
            bias_t = const_pool.tile([P, OT], mybir.dt.float32)
            if full_t:
                nc.sync.dma_start(
                    scale_t[:, :full_t], scale[: full_t * P].rearrange("(t p) -> p t", p=P)
                )
                nc.sync.dma_start(
                    bias_t[:, :full_t], bias[: full_t * P].rearrange("(t p) -> p t", p=P)
                )
            if rem:
                nc.sync.dma_start(
                    scale_t[:rem, full_t:], scale[full_t * P:].rearrange("(t p) -> p t", p=rem)
                )
                nc.sync.dma_start(
                    bias_t[:rem, full_t:], bias[full_t * P:].rearrange("(t p) -> p t", p=rem)
                )

            # PSUM bank groups: 4+4+3 o-tiles so two adjacent groups fit in
            # the 8 banks and group transitions never wait on drains.
            groups = []
            g0 = 0
            for gsz in (4, 4, 3):
                if g0 < OT:
                    groups.append((g0, min(g0 + gsz, OT)))
                    g0 += gsz

            def emit_conversions(sc):
                s0 = sc * S_CHUNK_
                his, los = [], []
                for kt in range(KT):
                    xstage = xstage_pool.tile([P, S_CHUNK_], mybir.dt.float32)
                    nc.sync.dma_start(xstage[:], xt[kt * P:(kt + 1) * P, s0:s0 + S_CHUNK_])
                    xhi = xhilo_pool.tile([P, S_CHUNK_], mybir.dt.bfloat16, tag="xhi")
                    nc.vector.tensor_copy(xhi[:], xstage[:])
                    xlo = xhilo_pool.tile([P, S_CHUNK_], mybir.dt.bfloat16, tag="xlo")
                    nc.vector.tensor_sub(xlo[:], xstage[:], xhi[:])
                    his.append(xhi)
                    los.append(xlo)
                    if sc == 0:
                        emit_w(kt)
                return his, los

            def emit_groups(sc, his, los):
                # kt outer / o-tile inner: each x tile's last reader comes
                # early in the group sweep, so next-chunk conversions spread
                # over the whole chunk instead of bunching at its tail.
                s0 = sc * S_CHUNK_
                for g_start, g_end in groups:
                    psums = {}
                    for ot in range(g_start, g_end):
                        psums[ot] = psum_pool.tile(
                            [P, S_CHUNK_], mybir.dt.float32,
                            name=f"psum_{sc}_{ot}", tag="psum",
                        )
                    for kt in range(KT):
                        for ot in range(g_start, g_end):
                            orows = min(P, O_SHARD_ - ot * P)
                            w_slice = w_res[kt][:, ot * P:ot * P + orows]
                            nc.tensor.matmul(
                                psums[ot][:orows, :], w_slice, his[kt][:],
                                start=(kt == 0), stop=False,
                            )
                            nc.tensor.matmul(
                                psums[ot][:orows, :], w_slice, los[kt][:],
                                start=False, stop=(kt == KT - 1),
                            )
                    for ot in range(g_start, g_end):
                        orows = min(P, O_SHARD_ - ot * P)
                        out_t = out_pool.tile([P, S_CHUNK_], mybir.dt.float32)
                        nc.vector.tensor_scalar(
                            out=out_t[:orows, :],
                            in0=psums[ot][:orows, :],
                            scalar1=scale_t[:orows, ot:ot + 1],
                            scalar2=bias_t[:orows, ot:ot + 1],
                            op0=mybir.AluOpType.mult,
                            op1=mybir.AluOpType.add,
                        )
                        nc.sync.dma_start(
                            yt[ot * P:ot * P + orows, s0:s0 + S_CHUNK_],
                            out_t[:orows, :],
                        )

            # Software-pipelined emission: conversions for chunk sc+1 are
            # emitted before chunk sc's matmul groups, so in the per-engine
            # FIFO streams next-chunk subs/casts sit ahead of this chunk's
            # PSUM drains.
            prev = emit_conversions(0)
            for sc in range(N_CHUNKS):
                if sc + 1 < N_CHUNKS:
                    nxt = emit_conversions(sc + 1)
                else:
                    nxt = None
                emit_groups(sc, *prev)
                prev = nxt

    nc.compile()
    return nc


_NC_CACHE = None


def _get_nc():
    global _NC_CACHE
    if _NC_CACHE is None:
        _NC_CACHE = build_bass()
    return _NC_CACHE


def run(inputs, trace=False, trace_cores=None, tmpdir=None):
    x = np.asarray(inputs["x"])
    w = np.asarray(inputs["weight_int8"])
    scale = np.asarray(inputs["scale"], dtype=np.float32)
    bias = np.asarray(inputs["bias"], dtype=np.float32)

    if w.dtype != np.int8:
        w = w.astype(np.int8)
    x2d = np.ascontiguousarray(x.reshape(S, I).astype(np.float32, copy=False))
    xtr = np.ascontiguousarray(x2d.T)  # [I, S]

    in_maps = []
    for c in range(N_CORES):
        sl = slice(c * O_SHARD, (c + 1) * O_SHARD)
        in_maps.append({
            "xt": xtr,
            "wt": np.ascontiguousarray(w[sl, :].T),  # [I, O_SHARD]
            "scale": np.ascontiguousarray(scale[sl]),
            "bias": np.ascontiguousarray(bias[sl]),
        })

    nc = _get_nc()
    kwargs = {}
    if trace:
        kwargs["trace"] = True
        if trace_cores is not None:
            kwargs["trace_cores"] = trace_cores
        if tmpdir is not None:
            kwargs["tmpdir"] = tmpdir
    res = run_bass_kernel_spmd(nc, in_maps, core_ids=list(range(N_CORES)), **kwargs)

    yt_full = np.concatenate([res.results[c]["yt"] for c in range(N_CORES)], axis=0)
    out = np.ascontiguousarray(yt_full.T).reshape(B, S, O).astype(np.float32, copy=False)
    if trace:
        return out, res
    return out


def kernel(**inputs) -> np.ndarray:
    return run(inputs, trace=False)


# revision 21
# speedup vs baseline: 1.0093x; 1.0041x over previous
"""CompressedLinear (int8 weight, per-row scale) on 8 Trainium2 NeuronCores.

Math: y[b,s,o] = sum_i x[b,s,i] * (w_int8[o,i] * scale[o]) + bias[o]

Strategy (tensor-parallel over out_features, per sharding hint):
  - Shard W/scale/bias rows across 8 cores (1376 rows each); x replicated.
  - Scale is applied to the matmul OUTPUT (algebraically identical), so the
    device matmuls run on the raw int8 weights cast to bf16 (int8 is exact
    in bf16).
  - x (f32) is split on-device into bf16 hi + lo parts (x = hi + lo with
    |x - hi - lo| <= 2^-16|x|), giving ~f32 accuracy from two bf16 matmul
    passes that accumulate into the same f32 PSUM tile - ~2x faster than
    fp32 matmuls on the PE.
  - Each core computes yT[o_shard, s] = W_shard @ x^T; both operands need
    the contraction dim on SBUF partitions, so the host hands each core
    pre-transposed views (pure layout permutation done while sharding):
    xt = x^T [4096, 2048] f32 and wt = W_shard^T [4096, 1376] int8.
  - Per-partition affine (scale, bias) is fused into the PSUM eviction.
"""

import os
import numpy as np

import concourse.bass as bass
import concourse.tile as tile
from concourse import bacc, mybir
from concourse.bass_utils import run_bass_kernel_spmd

B = 1
S = 2048
I = 4096
O = 11008
N_CORES = 8
O_SHARD = O // N_CORES  # 1376
S_CHUNK = 512
P = 128


def build_bass(I_=I, O_SHARD_=O_SHARD, S_=S, S_CHUNK_=S_CHUNK):
    KT = I_ // P
    N_CHUNKS = S_ // S_CHUNK_
    OT = (O_SHARD_ + P - 1) // P
    full_t = O_SHARD_ // P
    rem = O_SHARD_ - full_t * P

    nc = bacc.Bacc("TRN2", target_bir_lowering=False, debug=False)

    xt = nc.dram_tensor("xt", [I_, S_], mybir.dt.float32, kind="ExternalInput").ap()
    wt = nc.dram_tensor("wt", [I_, O_SHARD_], mybir.dt.int8, kind="ExternalInput").ap()
    scale = nc.dram_tensor("scale", [O_SHARD_], mybir.dt.float32, kind="ExternalInput").ap()
    bias = nc.dram_tensor("bias", [O_SHARD_], mybir.dt.float32, kind="ExternalInput").ap()
    yt = nc.dram_tensor("yt", [O_SHARD_, S_], mybir.dt.float32, kind="ExternalOutput").ap()

    with tile.TileContext(nc) as tc:
        with (
            tc.tile_pool(name="wres", bufs=1) as wres_pool,
            tc.tile_pool(name="consts", bufs=1) as const_pool,
            tc.tile_pool(name="xstage", bufs=4) as xstage_pool,
            tc.tile_pool(name="xhilo", bufs=min(KT + 8, KT * N_CHUNKS)) as xhilo_pool,
            tc.tile_pool(name="outp", bufs=4) as out_pool,
            tc.tile_pool(name="psum", bufs=8, space="PSUM") as psum_pool,
        ):
            # Weight shard int8 -> bf16, kept resident in SBUF. One tile per
            # k-slice so matmuls only depend on their own slice. The
            # int8->bf16 cast happens inside the DMA (SWDGE path), so no
            # compute engine spends time on it. Tiles are emitted interleaved
            # with the first chunk's x loads (see below) so kt=0 completes
            # first and matmuls start as early as possible.
            w_res = [None] * KT

            def emit_w(kt):
                w_kt = wres_pool.tile([P, O_SHARD_], mybir.dt.bfloat16, tag=f"w{kt}")
                nc.gpsimd.dma_start(w_kt[:], wt[kt * P:(kt + 1) * P, :])
                w_res[kt] = w_kt

            # PE warm-up: ~36 dependency-free matmuls on a zeroed tile keep
            # the PE busy during the initial DMA window, so the HAM clock
            # gate opens (K=8/8) before the first real matmul issues.
            warm_sb = const_pool.tile([P, P], mybir.dt.bfloat16)
            nc.any.memset(warm_sb[:], 0.0)
            warm_ps = psum_pool.tile([P, P], mybir.dt.float32, name="warm_ps", tag="psum")
            N_WARM = 36
            for i in range(N_WARM):
                nc.tensor.matmul(
                    warm_ps[:], warm_sb[:], warm_sb[:],
                    start=(i == 0), stop=(i == N_WARM - 1),
                )

            # per-partition scale/bias columns: [p, t] = value for o = t*128 + p
            scale_t = const_pool.tile([P, OT], mybir.dt.float32)
            bias_t = const_pool.tile([P, OT], mybir.dt.float32)
            if full_t:
                nc.sync.dma_start(
                    scale_t[:, :full_t], scale[: full_t * P].rearrange("(t p) -> p t", p=P)
                )
                nc.sync.dma_start(
                    bias_t[:, :full_t], bias[: full_t * P].rearrange("(t p) -> p t", p=P)
                )
            if rem:
                nc.sync.dma_start(
                    scale_t[:rem, full_t:], scale[full_t * P:].rearrange("(t p) -> p t", p=rem)
                )
                nc.sync.dma_start(
                    bias_t[:rem, full_t:], bias[full_t * P:].rearrange("(t p) -> p t", p=rem)
                )

            # PSUM bank groups: 4+4+3 o-tiles so two adjacent groups fit in
            # the 8 banks and group transitions never wait on drains.
            groups = []
            g0 = 0
            for gsz in (4, 4, 3):
                if g0 < OT:
                    groups.append((g0, min(g0 + gsz, OT)))
                    g0 += gsz

            def emit_conversions(sc):
                s0 = sc * S_CHUNK_
                his, los, casts = [], [], []
                for kt in range(KT):
                    xstage = xstage_pool.tile([P, S_CHUNK_], mybir.dt.float32)
                    nc.sync.dma_start(xstage[:], xt[kt * P:(kt + 1) * P, s0:s0 + S_CHUNK_])
                    xhi = xhilo_pool.tile([P, S_CHUNK_], mybir.dt.bfloat16, tag="xhi")
                    casts.append(nc.vector.tensor_copy(xhi[:], xstage[:]))
                    xlo = xhilo_pool.tile([P, S_CHUNK_], mybir.dt.bfloat16, tag="xlo")
                    nc.vector.tensor_sub(xlo[:], xstage[:], xhi[:])
                    his.append(xhi)
                    los.append(xlo)
                    if sc == 0 and kt < 2:
                        emit_w(kt)
                if sc == 0:
                    # Pace the remaining weight DMAs behind the chunk-0 x
                    # conversions: the x tiles are the startup critical path
                    # (PE consumes one every ~1.7us), and an unpaced 17MB
                    # weight flood shares SDMA packet round-robin with them,
                    # delaying every x completion.
                    for kt in range(2, KT):
                        w_kt = wres_pool.tile(
                            [P, O_SHARD_], mybir.dt.bfloat16, tag=f"w{kt}",
                            name=f"w_res_{kt}",
                        )
                        wd = nc.gpsimd.dma_start(w_kt[:], wt[kt * P:(kt + 1) * P, :])
                        bass._add_dep_helper(
                            wd.ins, casts[kt - 2].ins, sync=True,
                            reason="pace W DMAs behind startup x conversions",
                        )
                        w_res[kt] = w_kt
                return his, los

            def emit_groups(sc, his, los):
                # kt outer / o-tile inner: each x tile's last reader comes
                # early in the group sweep, so next-chunk conversions spread
                # over the whole chunk instead of bunching at its tail.
                s0 = sc * S_CHUNK_
                for g_start, g_end in groups:
                    psums = {}
                    for ot in range(g_start, g_end):
                        psums[ot] = psum_pool.tile(
                            [P, S_CHUNK_], mybir.dt.float32,
                            name=f"psum_{sc}_{ot}", tag="psum",
                        )
                    for kt in range(KT):
                        for ot in range(g_start, g_end):
                            orows = min(P, O_SHARD_ - ot * P)
                            w_slice = w_res[kt][:, ot * P:ot * P + orows]
                            nc.tensor.matmul(
                                psums[ot][:orows, :], w_slice, his[kt][:],
                                start=(kt == 0), stop=False,
                            )
                            nc.tensor.matmul(
                                psums[ot][:orows, :], w_slice, los[kt][:],
                                start=False, stop=(kt == KT - 1),
                            )
                    for ot in range(g_start, g_end):
                        orows = min(P, O_SHARD_ - ot * P)
                        out_t = out_pool.tile([P, S_CHUNK_], mybir.dt.float32)
                        nc.vector.tensor_scalar(
                            out=out_t[:orows, :],
                            in0=psums[ot][:orows, :],
                            scalar1=scale_t[:orows, ot:ot + 1],
                            scalar2=bias_t[:orows, ot:ot + 1],
                            op0=mybir.AluOpType.mult,
                            op1=mybir.AluOpType.add,
                        )
                        nc.sync.dma_start(
                            yt[ot * P:ot * P + orows, s0:s0 + S_CHUNK_],
                            out_t[:orows, :],
                        )

            # Software-pipelined emission: conversions for chunk sc+1 are
            # emitted before chunk sc's matmul groups, so in the per-engine
            # FIFO streams next-chunk subs/casts sit ahead of this chunk's
            # PSUM drains.
            prev = emit_conversions(0)
            for sc in range(N_CHUNKS):
                if sc + 1 < N_CHUNKS:
                    nxt = emit_conversions(sc + 1)
                else:
                    nxt = None
                emit_groups(sc, *prev)
                prev = nxt

    nc.compile()
    return nc


_NC_CACHE = None


def _get_nc():
    global _NC_CACHE
    if _NC_CACHE is None:
        _NC_CACHE = build_bass()
    return _NC_CACHE


def run(inputs, trace=False, trace_cores=None, tmpdir=None):
    x = np.asarray(inputs["x"])
    w = np.asarray(inputs["weight_int8"])
    scale = np.asarray(inputs["scale"], dtype=np.float32)
    bias = np.asarray(inputs["bias"], dtype=np.float32)

    if w.dtype != np.int8:
        w = w.astype(np.int8)
    x2d = np.ascontiguousarray(x.reshape(S, I).astype(np.float32, copy=False))
    xtr = np.ascontiguousarray(x2d.T)  # [I, S]

    in_maps = []
    for c in range(N_CORES):
        sl = slice(c * O_SHARD, (c + 1) * O_SHARD)
        in_maps.append({
            "xt": xtr,
            "wt": np.ascontiguousarray(w[sl, :].T),  # [I, O_SHARD]
            "scale": np.ascontiguousarray(scale[sl]),
            "bias": np.ascontiguousarray(bias[sl]),
        })

    nc = _get_nc()
    kwargs = {}
    if trace:
        kwargs["trace"] = True
        if trace_cores is not None:
            kwargs["trace_cores"] = trace_cores
        if tmpdir is not None:
            kwargs["tmpdir"] = tmpdir
    res = run_bass_kernel_spmd(nc, in_maps, core_ids=list(range(N_CORES)), **kwargs)

    yt_full = np.concatenate([res.results[c]["yt"] for c in range(N_CORES)], axis=0)
    out = np.ascontiguousarray(yt_full.T).reshape(B, S, O).astype(np.float32, copy=False)
    if trace:
        return out, res
    return out


def kernel(**inputs) -> np.ndarray:
    return run(inputs, trace=False)
